# revision 2
# baseline (speedup 1.0000x reference)
"""BitLinear (ternary-weight quantized linear) Trainium2 kernel.

Math (matches reference):
    delta  = mean(|W|) + 1e-5                    (global scalar)
    Wq     = clip(round(W/delta), -1, 1)         (ternary {-1,0,1})
    gamma  = max(|x|, axis=-1) + 1e-5            (per token row)
    k      = round(127*x/gamma)                  (integers in [-127,127])
    out    = (k @ Wq.T) * delta/127

Key facts exploited:
  * k and Wq are exact in bf16, so the 275-GFLOP matmul runs on the PE in
    bf16 with exact fp32 PSUM accumulation (|sum| <= 4096*127 < 2^24).
  * Rounding uses the fp32 magic-number trick (x + 1.5*2^23 - 1.5*2^23),
    which is round-to-nearest-even == jnp.round semantics.
  * W is passed host-side pre-transposed (W^T, layout [i, o]) so the
    contraction dim lands on SBUF partitions with no on-device W transpose.
  * x is quantized in natural layout (row max = free-axis reduce) and the
    small per-core x_q shard is transposed on the PE via identity matmuls.

Sharding: data-parallel over the 8192 token rows (1024 rows/core); W^T is
replicated.  delta needs a global abs-sum, computed by a tiny first launch
(each core reduces 1/8 of W), combined on host (the "all-reduce"), and fed
to the main launch as a broadcast scalar.
"""

import numpy as np
from contextlib import ExitStack

import concourse.bass as bass
import concourse.bacc as bacc
import concourse.tile as tile
import concourse.mybir as mybir
from concourse import masks
from concourse.bass_utils import run_bass_kernel_spmd

FP32 = mybir.dt.float32
BF16 = mybir.dt.bfloat16
ALU = mybir.AluOpType
AF = mybir.ActivationFunctionType
AX = mybir.AxisListType

N_CORES = 8
B, S, I = 4, 2048, 4096
O = 4096
R = B * S                    # 8192 token rows
RS = R // N_CORES            # 1024 rows per core
EPS = 1e-5
MAGIC = 12582912.0           # 1.5 * 2**23: fp32 round-to-nearest-even trick
KT = I // 128                # 32 contraction tiles
MT = RS // 128               # 8 row tiles per core
NT = O // 512                # 8 output-column blocks
W_SLICE = I // N_CORES       # 512 W^T rows per core for the delta pass


def _new_nc():
    return bacc.Bacc(
        "TRN2",
        target_bir_lowering=False,
        debug=False,
        enable_asserts=True,
        num_devices=N_CORES,
    )


def build_delta_nc():
    """Per-core partial abs-sum over a [512, 4096] slice of W."""
    nc = _new_nc()
    ws = nc.dram_tensor("ws", [W_SLICE, I], FP32, kind="ExternalInput").ap()
    partial = nc.dram_tensor("partial", [128, 1], FP32, kind="ExternalOutput").ap()

    with tile.TileContext(nc) as tc, ExitStack() as ctx:
        pool = ctx.enter_context(tc.tile_pool(name="ld", bufs=2))
        spool = ctx.enter_context(tc.tile_pool(name="st", bufs=1))
        pspool = ctx.enter_context(tc.tile_pool(name="ps", bufs=1, space="PSUM"))

        acc = spool.tile([128, 128], FP32)
        ones = spool.tile([128, 1], FP32)
        nc.vector.memset(ones, 1.0)

        for t in range(W_SLICE // 128):
            wtl = pool.tile([128, I], FP32, tag="wtl")
            nc.sync.dma_start(wtl, ws[128 * t : 128 * (t + 1), :])
            # chunked abs-sum: [128, 32, 128] --sum over last--> [128, 32]
            nc.vector.tensor_reduce(
                acc[:, 32 * t : 32 * (t + 1)],
                wtl.rearrange("p (c k) -> p c k", c=32),
                axis=AX.X,
                op=ALU.add,
                apply_absolute_value=True,
            )
        ps = pspool.tile([128, 1], FP32)
        nc.tensor.matmul(ps, acc, ones, start=True, stop=True)
        outsb = spool.tile([128, 1], FP32)
        nc.scalar.copy(outsb, ps)
        nc.sync.dma_start(partial, outsb)
    nc.compile()
    return nc


def build_main_nc():
    """Main launch: quantize x shard + W^T, bf16 matmul, scale, store."""
    nc = _new_nc()
    xs = nc.dram_tensor("xs", [RS, I], FP32, kind="ExternalInput").ap()
    wt = nc.dram_tensor("wt", [I, O], FP32, kind="ExternalInput").ap()
    dsum = nc.dram_tensor("dsum", [128, 1], FP32, kind="ExternalInput").ap()
    out = nc.dram_tensor("out", [RS, O], FP32, kind="ExternalOutput").ap()

    with tile.TileContext(nc) as tc, ExitStack() as ctx:
        const_pool = ctx.enter_context(tc.tile_pool(name="const", bufs=1))
        xt_pool = ctx.enter_context(tc.tile_pool(name="xt", bufs=1))

        ident = const_pool.tile([128, 128], BF16)
        masks.make_identity(nc, ident)

        dsum_sb = const_pool.tile([128, 1], FP32)
        nc.sync.dma_start(dsum_sb, dsum)
        delta = const_pool.tile([128, 1], FP32)
        nc.vector.tensor_scalar(delta, dsum_sb, 1.0 / (I * O), EPS, ALU.mult, ALU.add)
        inv_delta = const_pool.tile([128, 1], FP32)
        nc.vector.reciprocal(inv_delta, delta)
        d127 = const_pool.tile([128, 1], FP32)
        nc.vector.tensor_scalar_mul(d127, delta, 1.0 / 127.0)

        # resident quantized-transposed activations: 32 x [128, 1024] bf16
        xt_tiles = [
            xt_pool.tile([128, RS], BF16, name=f"xt{k}", tag=f"xt{k}")
            for k in range(KT)
        ]

        # ---- Phase X: load, quantize, PE-transpose the x shard ----
        with ExitStack() as xctx:
            xpool = xctx.enter_context(tc.tile_pool(name="xload", bufs=2))
            tpool = xctx.enter_context(tc.tile_pool(name="xtmp", bufs=2))
            qpool = xctx.enter_context(tc.tile_pool(name="xq", bufs=2))
            gpool = xctx.enter_context(tc.tile_pool(name="gam", bufs=2))
            tpsum = xctx.enter_context(tc.tile_pool(name="tps", bufs=4, space="PSUM"))

            for m in range(MT):
                xtl = xpool.tile([128, I], FP32, tag="x")
                nc.sync.dma_start(xtl, xs[128 * m : 128 * (m + 1), :])
                gm = gpool.tile([128, 1], FP32, tag="gm")
                nc.vector.tensor_reduce(
                    gm, xtl, axis=AX.X, op=ALU.max, apply_absolute_value=True
                )
                gme = gpool.tile([128, 1], FP32, tag="gme")
                nc.vector.tensor_scalar_add(gme, gm, EPS)
                rec = gpool.tile([128, 1], FP32, tag="rec")
                nc.vector.reciprocal(rec, gme)
                sc = gpool.tile([128, 1], FP32, tag="sc")
                nc.vector.tensor_scalar_mul(sc, rec, 127.0)
                # t1 = x * (127/gamma) + MAGIC   (rounds to nearest even)
                t1 = tpool.tile([128, I], FP32, tag="t1")
                nc.vector.tensor_scalar(t1, xtl, sc, MAGIC, ALU.mult, ALU.add)
                # xq = t1 - MAGIC  -> integer k, exact in bf16
                xq = qpool.tile([128, I], BF16, tag="xq")
                nc.scalar.activation(xq, t1, AF.Copy, bias=-MAGIC, scale=1.0)
                # transpose each 128x128 block onto the resident xt tiles
                for k in range(KT):
                    pst = tpsum.tile([128, 128], BF16, tag="pst")
                    nc.tensor.transpose(pst, xq[:, 128 * k : 128 * (k + 1)], ident)
                    nc.scalar.copy(xt_tiles[k][:, 128 * m : 128 * (m + 1)], pst)

        # ---- Phase MM: stream W^T, quantize to ternary bf16, matmul ----
        with ExitStack() as mctx:
            wpool = mctx.enter_context(tc.tile_pool(name="wload", bufs=4))
            w1pool = mctx.enter_context(tc.tile_pool(name="w1", bufs=2))
            w2pool = mctx.enter_context(tc.tile_pool(name="w2", bufs=2))
            wqpool = mctx.enter_context(tc.tile_pool(name="wq", bufs=4))
            opool = mctx.enter_context(tc.tile_pool(name="ost", bufs=4))
            mpsum = mctx.enter_context(tc.tile_pool(name="mps", bufs=1, space="PSUM"))

            for n in range(NT):
                psums = [
                    mpsum.tile([128, 512], FP32, name=f"ps{m}", tag=f"ps{m}")
                    for m in range(MT)
                ]
                for k in range(KT):
                    wtl = wpool.tile([128, 512], FP32, tag="w")
                    nc.sync.dma_start(
                        wtl, wt[128 * k : 128 * (k + 1), 512 * n : 512 * (n + 1)]
                    )
                    # r = W/delta + MAGIC  (rounded to int by fp32 arithmetic)
                    w1 = w1pool.tile([128, 512], FP32, tag="w1")
                    nc.vector.tensor_scalar(w1, wtl, inv_delta, MAGIC, ALU.mult, ALU.add)
                    # clip to MAGIC +- 1  (== clip(round(W/delta), -1, 1))
                    w2 = w2pool.tile([128, 512], FP32, tag="w2")
                    nc.vector.tensor_scalar(w2, w1, MAGIC + 1.0, MAGIC - 1.0, ALU.min, ALU.max)
                    # subtract MAGIC -> ternary, cast bf16
                    wq = wqpool.tile([128, 512], BF16, tag="wq")
                    nc.scalar.activation(wq, w2, AF.Copy, bias=-MAGIC, scale=1.0)
                    for m in range(MT):
                        nc.tensor.matmul(
                            psums[m],
                            xt_tiles[k][:, 128 * m : 128 * (m + 1)],
                            wq,
                            start=(k == 0),
                            stop=(k == KT - 1),
                        )
                for m in range(MT):
                    ob = opool.tile([128, 512], FP32, tag="ob")
                    nc.scalar.activation(ob, psums[m], AF.Copy, bias=0.0, scale=d127)
                    nc.sync.dma_start(
                        out[128 * m : 128 * (m + 1), 512 * n : 512 * (n + 1)], ob
                    )
    nc.compile()
    return nc


_NC_CACHE = {}


def _get_nc(name):
    if name not in _NC_CACHE:
        _NC_CACHE[name] = build_delta_nc() if name == "delta" else build_main_nc()
    return _NC_CACHE[name]


def kernel(x: np.ndarray, weight: np.ndarray) -> np.ndarray:
    x = np.asarray(x, dtype=np.float32)
    weight = np.asarray(weight, dtype=np.float32)
    core_ids = list(range(N_CORES))

    # host-side staging: W^T so the contraction dim is DMA-partition-major
    wtT = np.ascontiguousarray(weight.T)

    # ---- launch 1: per-core partial abs-sums over 1/8 of W ----
    nc_d = _get_nc("delta")
    in_maps_d = [
        {"ws": np.ascontiguousarray(wtT[c * W_SLICE : (c + 1) * W_SLICE, :])}
        for c in core_ids
    ]
    res_d = run_bass_kernel_spmd(nc_d, in_maps_d, core_ids)
    S_total = np.float64(0.0)
    for r in res_d.results:
        S_total += r["partial"].astype(np.float64).sum()
    dsum = np.full((128, 1), np.float32(S_total), dtype=np.float32)

    # ---- launch 2: main quantize + matmul ----
    nc_m = _get_nc("main")
    xf = np.ascontiguousarray(x.reshape(R, I))
    in_maps = [
        {
            "xs": np.ascontiguousarray(xf[c * RS : (c + 1) * RS, :]),
            "wt": wtT,
            "dsum": dsum,
        }
        for c in core_ids
    ]
    res_m = run_bass_kernel_spmd(nc_m, in_maps, core_ids)
    outs = [res_m.results[c]["out"] for c in core_ids]
    return np.concatenate(outs, axis=0).reshape(B, S, O)


if __name__ == "__main__":
    rng = np.random.default_rng(0)
    x = rng.standard_normal((B, S, I), dtype=np.float32)
    w = rng.standard_normal((O, I), dtype=np.float32)
    out = kernel(x, w)
    print("out shape", out.shape, "mean", out.mean(), "std", out.std())
